# revision 1
# baseline (speedup 1.0000x reference)
"""Trainium2 Bass kernel for the LogRatio loss (nn_LogRatio_14104672600201).

Math: the reference loss factorizes. Every masked reduction over
logsim[j, l] = log((X @ X.T)[j, l] + eps) has a mask that depends on j only
through targets[j] in [0, 64). So each row-reduction becomes a small GEMM
logsim @ Q with Q[l, g] built on host from labels, followed by a per-row
one-hot select at g = targets[j]:

  Q columns: [ P (one-hot of t_l) | W0T | W1T ]  (192 cols, zero-padded to 256)
  X1[j, g] = sum_l logsim[j, l]   * Q[l, g]
  X2[j, g] = sum_l logsim[j, l]^2 * Q[l, g]

  S1 = X1[j, t_j (P)] - diag_j          S2 = X2[j, t_j (P)] - diag_j^2
  A0 = H0[t_j]                          A1 = X1[j, t_j (W0)] + 0.1 * H1[t_j]
  A2 = X2[j, t_j (W0)] + 0.2 * X1[j, t_j (W1)] + 0.01 * H2[t_j]
  c  = cnt[t_j] - 1                     diag_j = log(||x_j||^2 + eps)
  loss = sum_j  S2 * A0 - 2 * S1 * A1 + c * A2

Sharding: data-parallel over j across 8 cores (512 rows each). Every core
holds the full xT (needed for the l dimension anyway) and computes its slab
of sim as [l_tile(128) x j_half(256)] PSUM tiles, so the reduction matmuls
get logsim already l-major (no on-chip transpose). Per-j partial losses are
DMA'd out and summed on host (the "all-reduce" of a scalar).

All matmuls use float32r (1 cycle/row at N >= 256, ~12-bit mantissa). To
keep that rounding harmless, the kernel works on the SHIFTED log
y = logsim - s (s ~ 3.5, folded into the Ln as Ln(sim * e^-s), so y is in
[-0.25, 0.45] and rounds ~10x finer). The shift terms are reconstructed
exactly in the epilogue from host-side tables:
  sum w*ls   = sum w*y  + s*sum(w)
  sum w*ls^2 = sum w*y^2 + 2s*sum(w*y) + s^2*sum(w)
"""

import numpy as np

N, D, KK, C = 4096, 128, 4, 64
NCORES = 8
JSH = N // NCORES          # 512 j rows per core
JH = 2                     # j-halves per core (256 cols each)
JCH = 4                    # j-chunks of 128 per core
LT = N // 128              # 32 l-tiles
QW = 256                   # padded Q width
NTAB = 6                   # tab columns in aux
EPS = 1e-6
OMEGA = 0.1
KSC = float(np.float32(np.exp(-3.5)))        # Ln input scale (exactly f32)
SHIFT = float(-np.log(np.float64(KSC)))      # effective shift s = -ln(KSC)

_CACHE = {}


def _build_nc():
    import bass_rust
    import concourse.bass as bass
    import concourse.bacc as bacc
    import concourse.mybir as mybir
    import concourse.tile as tile
    from contextlib import ExitStack

    f32 = mybir.dt.float32
    f32r = mybir.dt.float32r
    Ln = mybir.ActivationFunctionType.Ln
    mult = mybir.AluOpType.mult
    add = mybir.AluOpType.add
    AxX = mybir.AxisListType.X

    nc = bacc.Bacc("TRN2", target_bir_lowering=False, debug=False)
    xt = nc.dram_tensor("xt", [D, N], f32r, kind="ExternalInput")
    q = nc.dram_tensor("q", [LT, 128, QW], f32r, kind="ExternalInput")
    # aux columns: [0:64] one-hot of t_j, [64:70] tables, [70:198] x rows
    aux = nc.dram_tensor("aux", [JSH, C + NTAB + D], f32, kind="ExternalInput")
    lout = nc.dram_tensor("lout", [128, JCH], f32, kind="ExternalOutput")

    with tile.TileContext(nc) as tc, ExitStack() as ctx:
        cpool = ctx.enter_context(tc.tile_pool(name="const", bufs=1))
        lsp = ctx.enter_context(tc.tile_pool(name="lsp", bufs=1))
        work = ctx.enter_context(tc.tile_pool(name="work", bufs=4))
        small = ctx.enter_context(tc.tile_pool(name="small", bufs=2))
        psim = ctx.enter_context(tc.tile_pool(name="psim", bufs=3, space="PSUM"))
        px = ctx.enter_context(tc.tile_pool(name="px", bufs=1, space="PSUM"))

        # ---- constants: xT and Q resident in SBUF ----
        xt_sb = cpool.tile([D, N], f32r, tag="xt")
        for cchunk in range(4):
            sl = bass.ts(cchunk, 1024)
            nc.sync.dma_start(xt_sb[:, sl], xt[:, sl])
        q_sb = []
        q_dma = []
        for lt in range(LT):
            qt = cpool.tile([128, QW], f32r, tag=f"q{lt}", name=f"q_sb{lt}")
            q_dma.append(nc.sync.dma_start(qt[:], q[lt]))
            q_sb.append(qt)

        lbuf = cpool.tile([128, JCH], f32, tag="lbuf")

        for jh in range(JH):
            x1p = [
                px.tile([128, QW], f32, tag=f"x1_{i}", name=f"x1_{jh}_{i}")
                for i in range(2)
            ]
            x2p = [
                px.tile([128, QW], f32, tag=f"x2_{i}", name=f"x2_{jh}_{i}")
                for i in range(2)
            ]
            rhs_j = xt_sb[:, bass.ts(jh, 256)]
            for lt in range(LT):
                simp = psim.tile([128, 256], f32)
                nc.tensor.matmul(
                    simp[:],
                    xt_sb[:, bass.ts(lt, 128)],
                    rhs_j,
                    start=True,
                    stop=True,
                )
                ls = work.tile([128, 256], f32r, tag="ls")
                nc.scalar.activation(ls[:], simp[:], Ln, scale=KSC)
                ls2 = work.tile([128, 256], f32r, tag="ls2")
                nc.vector.tensor_mul(ls2[:], ls[:], ls[:])
                qr_ = q_sb[lt][:]
                for jc01 in range(2):
                    sl = bass.ts(jc01, 128)
                    nc.tensor.matmul(
                        x2p[jc01][:], ls2[:, sl], qr_,
                        start=(lt == 0), stop=(lt == LT - 1),
                    )
                    nc.tensor.matmul(
                        x1p[jc01][:], ls[:, sl], qr_,
                        start=(lt == 0), stop=(lt == LT - 1),
                    )

            # ---- epilogue per 128-row j-chunk ----
            for jc01 in range(2):
                jc = jh * 2 + jc01
                jsl = bass.ts(jc, 128)
                aux_t = small.tile(
                    [128, C + NTAB + D], f32, tag=f"aux{jc}", name=f"aux{jc}"
                )
                nc.sync.dma_start(aux_t[:], aux[jsl, :])
                pj_t = aux_t[:, 0:C]
                tab_t = aux_t[:, C : C + NTAB]
                xj_t = aux_t[:, C + NTAB : C + NTAB + D]

                sels = []
                for name, src in (
                    ("s1p", x1p[jc01][:, 0:64]),
                    ("s1w0", x1p[jc01][:, 64:128]),
                    ("s1w1", x1p[jc01][:, 128:192]),
                    ("s2p", x2p[jc01][:, 0:64]),
                    ("s2w0", x2p[jc01][:, 64:128]),
                ):
                    scr = small.tile(
                        [128, C], f32, tag=f"scr{jc}_{name}", name=f"scr_{jc}_{name}"
                    )
                    sel = small.tile(
                        [128, 1], f32, tag=f"{name}{jc}", name=f"{name}_{jc}"
                    )
                    nc.vector.tensor_mul(scr[:], src, pj_t)
                    nc.vector.reduce_sum(sel[:], scr[:], axis=AxX)
                    sels.append(sel)
                s1p, s1w0, s1w1, s2p, s2w0 = sels

                scr2 = small.tile([128, D], f32, tag=f"scr2{jc}", name=f"scr2{jc}")
                nrm = small.tile([128, 1], f32, tag=f"nrm{jc}", name=f"nrm{jc}")
                nc.vector.tensor_mul(scr2[:], xj_t, xj_t)
                nc.vector.reduce_sum(nrm[:], scr2[:], axis=AxX)
                diag = small.tile([128, 1], f32, tag=f"diag{jc}", name=f"diag{jc}")
                nc.scalar.activation(diag[:], nrm[:], Ln)
                diag2 = small.tile([128, 1], f32, tag=f"diag2{jc}", name=f"diag2{jc}")
                nc.vector.tensor_mul(diag2[:], diag[:], diag[:])

                # shift reconstruction: sels are y / y^2 sums, y = ls - s
                # S1 = yP + s*cnt - diag
                s1 = small.tile([128, 1], f32, tag=f"s1{jc}", name=f"s1{jc}")
                nc.vector.tensor_add(s1[:], s1p[:], tab_t[:, 2:3])
                nc.vector.tensor_sub(s1[:], s1[:], diag[:])
                # S2 = y2P + 2s*yP + s^2*cnt - diag^2
                s2 = small.tile([128, 1], f32, tag=f"s2{jc}", name=f"s2{jc}")
                nc.vector.scalar_tensor_tensor(
                    out=s2[:], in0=s1p[:], scalar=2.0 * SHIFT, in1=s2p[:],
                    op0=mult, op1=add,
                )
                nc.vector.tensor_add(s2[:], s2[:], tab_t[:, 3:4])
                nc.vector.tensor_sub(s2[:], s2[:], diag2[:])
                # A1 = yW0 + (s*H0 + 0.1*H1)
                a1 = small.tile([128, 1], f32, tag=f"a1{jc}", name=f"a1{jc}")
                nc.vector.tensor_add(a1[:], s1w0[:], tab_t[:, 4:5])
                # A2 = y2W0 + 2s*yW0 + 0.2*yW1 + (s^2*H0 + 0.2s*H1 + 0.01*H2)
                a2 = small.tile([128, 1], f32, tag=f"a2{jc}", name=f"a2{jc}")
                nc.vector.scalar_tensor_tensor(
                    out=a2[:], in0=s1w0[:], scalar=2.0 * SHIFT, in1=s2w0[:],
                    op0=mult, op1=add,
                )
                nc.vector.scalar_tensor_tensor(
                    out=a2[:], in0=s1w1[:], scalar=0.2, in1=a2[:],
                    op0=mult, op1=add,
                )
                nc.vector.tensor_add(a2[:], a2[:], tab_t[:, 5:6])
                # L = s2 * A0 + c * a2 - 2 * s1 * a1
                u = small.tile([128, 1], f32, tag=f"u{jc}", name=f"u{jc}")
                nc.vector.tensor_mul(u[:], s2[:], tab_t[:, 1:2])
                w = small.tile([128, 1], f32, tag=f"w{jc}", name=f"w{jc}")
                nc.vector.tensor_mul(w[:], a2[:], tab_t[:, 0:1])
                v = small.tile([128, 1], f32, tag=f"v{jc}", name=f"v{jc}")
                nc.vector.tensor_mul(v[:], s1[:], a1[:])
                nc.vector.tensor_add(u[:], u[:], w[:])
                nc.vector.scalar_tensor_tensor(
                    out=lbuf[:, jc : jc + 1], in0=v[:], scalar=-2.0,
                    in1=u[:], op0=mult, op1=add,
                )

        nc.sync.dma_start(lout[:], lbuf[:])
    nc.compile()
    return nc


def _host_prep(inputs, labels):
    x = np.ascontiguousarray(np.asarray(inputs, dtype=np.float32))
    lab = np.asarray(labels)
    t = lab[:, 0]

    m = np.arange(KK)
    om = np.float32(OMEGA)
    lp = (
        np.log(np.float32(OMEGA + EPS))
        - np.log((om ** (KK - m + 1)).astype(np.float32) + np.float32(EPS))
    ).astype(np.float32)

    gr = np.arange(C)
    eq = lab[None, :, :] == gr[:, None, None]          # [C, N, KK]
    nm = np.stack(
        [
            ~eq[:, :, 3],
            eq[:, :, 3] & ~eq[:, :, 2],
            eq[:, :, 2] & ~eq[:, :, 1],
            eq[:, :, 1] & ~eq[:, :, 0],
        ]
    ).astype(np.float32)                                # [KK, C, N]
    w0 = nm.sum(0)
    w1 = np.einsum("m,mcl->cl", lp, nm).astype(np.float32)
    w2 = np.einsum("m,mcl->cl", lp * lp, nm).astype(np.float32)
    ph = (t[:, None] == gr[None, :]).astype(np.float32)  # [N, C] one-hot t_l

    qm = np.zeros((N, QW), dtype=np.float32)
    qm[:, 0:C] = ph
    qm[:, C : 2 * C] = w0.T
    qm[:, 2 * C : 3 * C] = w1.T

    h0 = w0.sum(1)
    h1 = w1.sum(1)
    h2 = w2.sum(1)
    cnt = ph.sum(0)
    s = np.float64(SHIFT)
    tab = np.stack(
        [
            cnt[t] - 1.0,
            h0[t],
            s * cnt[t],
            s * s * cnt[t],
            s * h0[t] + 0.1 * h1[t],
            s * s * h0[t] + 0.2 * s * h1[t] + 0.01 * h2[t],
        ],
        axis=1,
    ).astype(np.float32)                                # [N, NTAB]

    xt = np.ascontiguousarray(x.T)                       # [D, N]
    auxf = np.concatenate([ph, tab, x], axis=1).astype(np.float32)  # [N, 198]
    in_maps = []
    for cid in range(NCORES):
        sl = slice(cid * JSH, (cid + 1) * JSH)
        # rotate the l axis so this core's own j-shard sits at columns
        # 0:JSH — the kernel always matmuls against xt[:, 0:512]; the l
        # reduction (over all 4096) is rotation-invariant as long as q's
        # rows rotate identically.
        xtc = np.ascontiguousarray(np.roll(xt, -cid * JSH, axis=1))
        qc = np.ascontiguousarray(
            np.roll(qm, -cid * JSH, axis=0).reshape(LT, 128, QW)
        )
        in_maps.append(
            {
                "xt": xtc,
                "q": qc,
                "aux": np.ascontiguousarray(auxf[sl]),
            }
        )
    return in_maps


def _run(inputs, labels, trace=False, tmpdir=None):
    from concourse.bass_utils import run_bass_kernel_spmd

    if "nc" not in _CACHE:
        _CACHE["nc"] = _build_nc()
    in_maps = _host_prep(inputs, labels)
    res = run_bass_kernel_spmd(
        _CACHE["nc"], in_maps, core_ids=list(range(NCORES)),
        trace=trace, tmpdir=tmpdir,
    )
    loss = np.float64(0.0)
    for r in res.results:
        loss += r["lout"].astype(np.float64).sum()
    return np.array(loss, dtype=np.float32), res


def kernel(inputs, labels):
    out, _ = _run(inputs, labels, trace=False)
    return out



# revision 4
# speedup vs baseline: 1.3029x; 1.3029x over previous
"""Trainium2 Bass kernel for the LogRatio loss (nn_LogRatio_14104672600201).

Math: the reference loss factorizes (see the derivation in the epilogue
below). Every masked reduction over logsim[j, l] = log((X @ X.T)[j, l]) has a
mask depending on j only through targets[j] in [0, 64), so each row-reduction
becomes a GEMM against a label-derived matrix Q[l, g] followed by a per-row
one-hot select at g = targets[j].

Layout (g-major): per core (512-row j-shard, rotated so its own rows sit at
columns 0:512 of the l axis):

  for each of 32 l-tiles:
    sim   = xT_tile.T @ xT[:, 0:512]            # [128 l, 512 j]  PSUM
    y     = Ln(KSC * sim)                       # shifted log, bf16
    y2    = y * y                               # DVE bf16 2x
    X1a  += Qa.T @ y      (Qa = [P | W0])       # [128 g, 512 j]  accum
    X2a  += Qa.T @ y2                           # [128 g, 512 j]  accum
    X1b  += Qb.T @ y      (Qb = [W1])           # [ 64 g, 512 j]  accum

Q is STATIONARY and the full j-shard is the MOVING operand (N=512), so the
PE runs ~134 big matmuls instead of 320 small ones, and LDWEIGHTS drops
3x. All matmul operands are bf16 (the shift trick keeps y in [-0.3, 0.35],
so bf16's 8-bit mantissa costs only ~1e-4 absolute per element).

Selection: the 5 per-j values (yP, yW0, yW1, y2P, y2W0) are extracted
without transposes: multiply the [g, j] accumulators elementwise by the
one-hot mask M[g, j] = (g % 64 == t_j), then contract over partitions with a
tiny 2-column indicator matmul -> [2, 512] rows. Those 6 rows DMA out and
the final scalar loss is reconstructed on host in float64.
"""

import numpy as np
import ml_dtypes

N, D, KK, C = 4096, 128, 4, 64
NCORES = 8
JSH = N // NCORES          # 512 j rows per core
LT = N // 128              # 32 l-tiles
GW = 192                   # Q width: [P(64) | W0(64) | W1(64)]
EPS = 1e-6
OMEGA = 0.1
KSC = float(np.float32(np.exp(-3.5)))        # Ln input scale (exactly f32)
SHIFT = float(-np.log(np.float64(KSC)))      # effective shift s = -ln(KSC)

_CACHE = {}


def _build_nc():
    import concourse.bass as bass
    import concourse.bacc as bacc
    import concourse.mybir as mybir
    import concourse.tile as tile
    from contextlib import ExitStack

    f32 = mybir.dt.float32
    f32r = mybir.dt.float32r
    bf16 = mybir.dt.bfloat16
    Ln = mybir.ActivationFunctionType.Ln

    nc = bacc.Bacc("TRN2", target_bir_lowering=False, debug=False)
    xt = nc.dram_tensor("xt", [D, N], bf16, kind="ExternalInput")
    # q[p, lt*GW + g] = Q[lt*128 + p, g]
    q = nc.dram_tensor("q", [128, LT * GW], bf16, kind="ExternalInput")
    # mask[g, j] = (g % 64 == t_j), stacked twice along g
    msk = nc.dram_tensor("msk", [128, JSH], bf16, kind="ExternalInput")
    # indicator: ind[g, 0] = (g < 64), ind[g, 1] = (g >= 64)
    ind = nc.dram_tensor("ind", [128, 2], bf16, kind="ExternalInput")
    # out rows: [yP, yW0 | yW1, 0 | y2P, y2W0]
    lout = nc.dram_tensor("lout", [2, 3 * JSH], f32, kind="ExternalOutput")

    with tile.TileContext(nc) as tc, ExitStack() as ctx:
        cpool = ctx.enter_context(tc.tile_pool(name="const", bufs=1))
        work = ctx.enter_context(tc.tile_pool(name="work", bufs=3))
        mpool = ctx.enter_context(tc.tile_pool(name="mpool", bufs=1))
        psim = ctx.enter_context(tc.tile_pool(name="psim", bufs=3, space="PSUM"))
        px = ctx.enter_context(tc.tile_pool(name="px", bufs=1, space="PSUM"))
        psel = ctx.enter_context(tc.tile_pool(name="psel", bufs=2, space="PSUM"))

        # ---- constants resident in SBUF ----
        xt_sb = cpool.tile([D, N], bf16, tag="xt")
        for ch in range(4):
            sl = bass.ts(ch, 1024)
            nc.sync.dma_start(xt_sb[:, sl], xt[:, sl])
        q_sb = cpool.tile([128, LT * GW], bf16, tag="q")
        for ch in range(6):
            sl = bass.ts(ch, 1024)
            nc.sync.dma_start(q_sb[:, sl], q[:, sl])
        msk_sb = cpool.tile([128, JSH], bf16, tag="msk")
        nc.sync.dma_start(msk_sb[:], msk[:])
        ind_sb = cpool.tile([128, 2], bf16, tag="ind")
        nc.sync.dma_start(ind_sb[:], ind[:])

        # accumulators (one PSUM bank each, held across the whole lt loop)
        x1a = px.tile([128, JSH], f32, tag="x1a")
        x2a = px.tile([128, JSH], f32, tag="x2a")
        x1b = px.tile([64, JSH], f32, tag="x1b")

        mov = xt_sb[:, 0:JSH]
        ls_t = [None] * LT
        ls2_t = [None] * LT

        def sim_stage(lt):
            simp = psim.tile([128, JSH], f32, tag="simp", name=f"simp{lt}")
            nc.tensor.matmul(
                simp[:], xt_sb[:, bass.ts(lt, 128)], mov, start=True, stop=True
            )
            ls = work.tile([128, JSH], bf16, tag="ls", name=f"ls{lt}")
            nc.scalar.activation(ls[:], simp[:], Ln, scale=KSC)
            ls2 = work.tile([128, JSH], bf16, tag="ls2", name=f"ls2{lt}")
            nc.vector.tensor_mul(ls2[:], ls[:], ls[:])
            ls_t[lt] = ls
            ls2_t[lt] = ls2

        sim_stage(0)
        sim_stage(1)
        for lt in range(LT):
            if lt + 2 < LT:
                sim_stage(lt + 2)
            qa = q_sb[:, lt * GW : lt * GW + 128]
            qb = q_sb[:, lt * GW + 128 : lt * GW + GW]
            st = lt == 0
            sp = lt == LT - 1
            nc.tensor.matmul(x1a[:], qa, ls_t[lt][:], start=st, stop=sp)
            nc.tensor.matmul(x2a[:], qa, ls2_t[lt][:], start=st, stop=sp)
            nc.tensor.matmul(x1b[:], qb, ls_t[lt][:], start=st, stop=sp)

        # ---- selection: mask-mul then 2-column collapse matmul ----
        sel_sb = mpool.tile([2, 3 * JSH], f32, tag="selsb")
        for i, (src, prange) in enumerate(((x1a, 128), (x1b, 64), (x2a, 128))):
            m = mpool.tile([prange, JSH], bf16, tag=f"m{i}", name=f"m{i}")
            nc.vector.tensor_mul(m[:], src[0:prange, :], msk_sb[0:prange, :])
            sel = psel.tile([2, JSH], f32, tag="sel", name=f"sel{i}")
            nc.tensor.matmul(
                sel[:], ind_sb[0:prange, :], m[:], start=True, stop=True
            )
            nc.vector.tensor_copy(sel_sb[:, bass.ts(i, JSH)], sel[:])
        nc.sync.dma_start(lout[:], sel_sb[:])
    nc.compile()
    return nc


def _host_prep(inputs, labels):
    x = np.asarray(inputs, dtype=np.float32)
    lab = np.asarray(labels)
    t = lab[:, 0].astype(np.int64)
    bf = ml_dtypes.bfloat16

    m = np.arange(KK)
    om = np.float64(OMEGA)
    lp = np.log(np.float64(OMEGA + EPS)) - np.log(om ** (KK - m + 1) + np.float64(EPS))

    gr = np.arange(C)
    eq = lab[None, :, :] == gr[:, None, None]          # [C, N, KK]
    nm = np.stack(
        [
            ~eq[:, :, 3],
            eq[:, :, 3] & ~eq[:, :, 2],
            eq[:, :, 2] & ~eq[:, :, 1],
            eq[:, :, 1] & ~eq[:, :, 0],
        ]
    ).astype(np.float64)                                # [KK, C, N]
    w0 = nm.sum(0)                                      # [C, N]
    w1 = np.einsum("m,mcl->cl", lp, nm)
    w2 = np.einsum("m,mcl->cl", lp * lp, nm)
    ph = (t[:, None] == gr[None, :]).astype(np.float64)  # [N, C] one-hot t_l

    qm = np.zeros((N, GW), dtype=np.float32)
    qm[:, 0:C] = ph
    qm[:, C : 2 * C] = w0.T
    qm[:, 2 * C : 3 * C] = w1.T

    ind = np.zeros((128, 2), dtype=np.float32)
    ind[0:64, 0] = 1.0
    ind[64:128, 1] = 1.0

    xt = np.ascontiguousarray(x.T)                       # [D, N]
    in_maps = []
    for cid in range(NCORES):
        sl = slice(cid * JSH, (cid + 1) * JSH)
        # rotate the l axis so this core's own j-shard sits at columns
        # 0:JSH; the l reduction (over all 4096) is rotation-invariant as
        # long as q's rows rotate identically.
        xtc = np.roll(xt, -cid * JSH, axis=1)
        qc = np.roll(qm, -cid * JSH, axis=0)             # [N, GW]
        # q_sb[p, lt*GW + g] = Q[lt*128 + p, g]
        qsb = np.ascontiguousarray(
            qc.reshape(LT, 128, GW).transpose(1, 0, 2).reshape(128, LT * GW)
        )
        oh = (gr[:, None] == t[sl][None, :]).astype(np.float32)  # [64, 512]
        mk = np.concatenate([oh, oh], axis=0)            # [128, 512]
        in_maps.append(
            {
                "xt": xtc.astype(bf),
                "q": qsb.astype(bf),
                "msk": mk.astype(bf),
                "ind": ind.astype(bf),
            }
        )

    tabs = {
        "t": t, "cnt": ph.sum(0), "h0": w0.sum(1), "h1": w1.sum(1),
        "h2": w2.sum(1), "x": x,
    }
    return in_maps, tabs


def _host_loss(res_list, tabs):
    t, cnt, h0, h1, h2 = tabs["t"], tabs["cnt"], tabs["h0"], tabs["h1"], tabs["h2"]
    x64 = tabs["x"].astype(np.float64)
    s = np.float64(SHIFT)
    loss = np.float64(0.0)
    for cid, r in enumerate(res_list):
        sl = slice(cid * JSH, (cid + 1) * JSH)
        lo = r["lout"].astype(np.float64)                # [2, 3*JSH]
        yP, yW0 = lo[0, 0:JSH], lo[1, 0:JSH]
        yW1 = lo[0, JSH : 2 * JSH]
        y2P, y2W0 = lo[0, 2 * JSH :], lo[1, 2 * JSH :]
        tj = t[sl]
        cj, h0j, h1j, h2j = cnt[tj], h0[tj], h1[tj], h2[tj]
        diag = np.log(np.einsum("jd,jd->j", x64[sl], x64[sl]) + EPS)
        S1 = yP + s * cj - diag
        S2 = y2P + 2 * s * yP + s * s * cj - diag * diag
        A1 = yW0 + s * h0j + 0.1 * h1j
        A2 = (y2W0 + 2 * s * yW0 + s * s * h0j) + 0.2 * (yW1 + s * h1j) + 0.01 * h2j
        loss += np.sum(S2 * h0j - 2.0 * S1 * A1 + (cj - 1.0) * A2)
    return np.array(loss, dtype=np.float32)


def _run(inputs, labels, trace=False, tmpdir=None):
    from concourse.bass_utils import run_bass_kernel_spmd

    if "nc" not in _CACHE:
        _CACHE["nc"] = _build_nc()
    in_maps, tabs = _host_prep(inputs, labels)
    res = run_bass_kernel_spmd(
        _CACHE["nc"], in_maps, core_ids=list(range(NCORES)),
        trace=trace, tmpdir=tmpdir,
    )
    return _host_loss(res.results, tabs), res


def kernel(inputs, labels):
    out, _ = _run(inputs, labels, trace=False)
    return out
